# revision 38
# baseline (speedup 1.0000x reference)
"""Trainium2 Bass kernel for sparse (top-k) multi-headed attention.

Problem shapes (hardcoded):
  x, source: [B=4, D=256, N=M=2048] f32
  Wq/Wk/Wv/Wm: [256, 256], bq/bk/bv/bm: [256], k=32 (top-k), H=4 heads, dim=64.

Sharding: 8 cores; core c handles batch b=c//2 and head pair hp=c%2
(heads 2hp, 2hp+1).  Channel c of D maps to (d, h) = (c//4, c%4) per the
reference reshape(B, dim, H, N).  The host reorders each core's 128
channels head-major/d-major so each head occupies 64 contiguous SBUF
partitions.  Each core returns its partial merge
  part = Wm[:, ch].T? -> out_part[o, n] = sum_{i in ch} Wm[o, i] * merged[i, n]
and the host sums the two partials per batch and adds bm.

Stage-skewed software pipeline, one slot per tile (tile = 128 query rows
x 2048 keys, 32 tiles/core); stage X(t-k) is emitted in slot t so every
engine stream stays dependency-clean (in-order engines, so a stage that
waits head-of-line-blocks everything behind it):
  slot t   PE : 4 fp32 score matmuls -> two [128,1024] PSUM tiles
           ACT: s_sb = copy(PSUM) per half (frees PSUM banks early)
           DVE: top-k peel = 4x (max8 + match_replace), exactly-k even
                with duplicate scores (match_replace replaces one
                instance per matched value)
  t-1      ACT: den = accum_out of exp(scale*m32) (one instruction)
  t-2      DVE: rden = 1/den
  t-3      ACT: nlden = Ln(rden); e = exp(scale*s + nlden) and
                emk = exp(scale*scratch + nlden), both fp16: identical
                LUT args except at the k top positions, so e - emk is
                exactly 0 off-support / the normalized weight on it
  t-4      GPS: pn = e - emk  (GpSimd, keeps it off the busy DVE/ACT)
  t-5      DMA: one batched XBAR transpose-DMA of pn into the per-phase
                pT layout [m_local, mt, n]  (replaces 512 PE transpose
                matmuls + 128 ACT PSUM->SBUF copies)
  fifo     PE : AV accumulation in fp16 (4 matmuls per slot), av psum
                double-buffered via the tps ring; merge matmul + output
                DMA per super-tile, phase-end copies deferred 2 slots so
                ACT never waits on PE's AV matmuls.
"""

import os
import sys

import ml_dtypes
import numpy as np

for _p in ("/opt/trn_rl_repo",):
    if _p not in sys.path and os.path.isdir(_p):
        sys.path.insert(0, _p)

import concourse.bass as bass
import concourse.mybir as mybir
import concourse.tile as tile
from concourse.bass_utils import run_bass_kernel_spmd
from concourse.masks import make_identity

B, D, N, M = 4, 256, 2048, 2048
H = 4
DIM = D // H  # 64
P = 128
NT = N // P  # n-tiles of 128 rows
MT = M // P  # m-tiles of 128 cols
SCALE = 1.0 / float(np.sqrt(DIM))  # 0.125
N_CORES = 8

FP = mybir.dt.float32
FR = mybir.dt.float32r
BF = mybir.dt.bfloat16
HF = mybir.dt.float16
A = mybir.AluOpType
AF = mybir.ActivationFunctionType


def _legalize_sync_waits(bir: dict) -> dict:
    """Split multi-wait instructions: walrus codegen allows only ONE sync wait
    per engine instruction (PE is HW-decoded; ACT/CTRL structs are just as
    limited).  Insert single-wait NoOps on the same engine immediately before
    any instruction carrying more than one wait; each NoOp takes one wait, the
    original keeps the last wait plus its updates."""
    nid = [0]
    for fn in bir["functions"]:
        for blk in fn["blocks"]:
            out = []
            for ins in blk["instructions"]:
                si = ins.get("sync_info")
                waits = (si or {}).get("on_wait") or []
                if len(waits) > 1:
                    for w in waits[:-1]:
                        nid[0] += 1
                        out.append(
                            {
                                "engine": ins["engine"],
                                "ins": [],
                                "name": f"{ins['name']}-sw{nid[0]}",
                                "opcode": "NoOp",
                                "outs": [],
                                "sync_info": {"on_update": [], "on_wait": [w]},
                            }
                        )
                    si["on_wait"] = [waits[-1]]
                out.append(ins)
            blk["instructions"] = out
    return bir


def build_program(k: int) -> bass.Bass:
    nrounds = (k + 7) // 8
    rem = k - (nrounds - 1) * 8  # valid slots in the last round (1..8)

    nc = bass.Bass(
        "TRN2",
        target_bir_lowering=False,
        debug=False,
        enable_asserts=True,
        num_devices=N_CORES,
    )

    # DRAM parameters (per-core shards, prepared by the host)
    xb = nc.dram_tensor("xb", [D, N], FP, kind="ExternalInput").ap()
    src = nc.dram_tensor("src", [D, M], FP, kind="ExternalInput").ap()
    wqT = nc.dram_tensor("wqT", [D, P], FP, kind="ExternalInput").ap()
    wkT = nc.dram_tensor("wkT", [D, P], FP, kind="ExternalInput").ap()
    wvT = nc.dram_tensor("wvT", [D, P], FP, kind="ExternalInput").ap()
    wmT = nc.dram_tensor("wmT", [P, D], FP, kind="ExternalInput").ap()
    bqv = nc.dram_tensor("bq", [P, 1], FP, kind="ExternalInput").ap()
    bkv = nc.dram_tensor("bk", [P, 1], FP, kind="ExternalInput").ap()
    bvv = nc.dram_tensor("bv", [P, 1], FP, kind="ExternalInput").ap()
    part = nc.dram_tensor("part", [D, N], FP, kind="ExternalOutput").ap()

    from contextlib import ExitStack

    with tile.TileContext(nc) as tc, ExitStack() as ctx:
        consts = ctx.enter_context(tc.tile_pool(name="consts", bufs=1))
        wpool = ctx.enter_context(tc.tile_pool(name="w", bufs=1))
        qkvp = ctx.enter_context(tc.tile_pool(name="qkv", bufs=1))
        vtp = ctx.enter_context(tc.tile_pool(name="vt", bufs=1))
        xpool_cm = tc.tile_pool(name="x", bufs=1)
        xpool = xpool_cm.__enter__()

        identity = consts.tile([P, P], FP)
        make_identity(nc, identity)

        # ---- load weights / biases / activations ----
        # k-path tensors on the Sync DGE queue, q-path on the Activation DGE
        # queue so the two transfer streams overlap; wv/wm later on Sync.
        w_tiles = {}
        for name, ap, eng in (
            ("wk", wkT, nc.sync), ("wq", wqT, nc.scalar), ("wv", wvT, nc.sync)
        ):
            t0 = wpool.tile([P, P], FP, tag=name + "0")
            t1 = wpool.tile([P, P], FP, tag=name + "1")
            eng.dma_start(out=t0[:], in_=ap[0:P, :])
            eng.dma_start(out=t1[:], in_=ap[P : 2 * P, :])
            w_tiles[name] = (t0, t1)
        wm_sb = wpool.tile([P, D], FP, tag="wm")
        nc.sync.dma_start(out=wm_sb[:], in_=wmT[:, :])
        b_tiles = {}
        for name, ap, eng in (
            ("bq", bqv, nc.scalar), ("bk", bkv, nc.sync), ("bv", bvv, nc.sync)
        ):
            t = wpool.tile([P, 1], FP, tag=name)
            eng.dma_start(out=t[:], in_=ap[:, :])
            b_tiles[name] = t

        x_sb = [xpool.tile([P, N], FP, tag=f"x{i}", name=f"x{i}") for i in range(2)]
        src_sb = [xpool.tile([P, M], FP, tag=f"s{i}", name=f"s{i}") for i in range(2)]
        for i in range(2):
            nc.scalar.dma_start(out=x_sb[i][:], in_=xb[i * P : (i + 1) * P, :])
            nc.sync.dma_start(out=src_sb[i][:], in_=src[i * P : (i + 1) * P, :])

        # Persistent PSUM pools (8 banks total, never recycled across phases):
        #   sps: one [P, 2048] fp32 score tile  = 4 banks
        #   tps: junk / vT transposes / p-transposes (bufs=2)
        #   avps: AV accumulator [64, 512]      = 1 bank
        #   mgps: merge matmul [128, 512]       = 1 bank
        sps = ctx.enter_context(tc.tile_pool(name="sps", bufs=2, space="PSUM"))
        tps = ctx.enter_context(tc.tile_pool(name="tps", bufs=2, space="PSUM"))
        mgps = ctx.enter_context(tc.tile_pool(name="mgps", bufs=2, space="PSUM"))

        # Absorb DMA-completion semaphores into PE's observed clock in three
        # stages so the k-projection starts as soon as its own inputs landed:
        # one tiny single-wait matmul per loaded tile, so no later PE
        # instruction needs a second fresh wait.
        stage_a = [w_tiles["wk"][0], w_tiles["wk"][1], b_tiles["bk"]] + src_sb
        stage_b = [w_tiles["wq"][0], w_tiles["wq"][1], b_tiles["bq"]] + x_sb
        stage_c = [
            w_tiles["wv"][0], w_tiles["wv"][1], b_tiles["bv"], wm_sb
        ]
        all_loaded = [identity] + stage_a + stage_b + stage_c
        HM0 = M // 2
        junk = tps.tile([P, P], FP, tag="ptp", name="junk")
        ji = [0]

        def junk_absorb(tiles):
            for t in tiles:
                nc.tensor.matmul(
                    junk[0:1, ji[0] : ji[0] + 1], lhsT=t[:, 0:1], rhs=t[:, 0:1],
                    start=True, stop=True, skip_group_check=True,
                )
                ji[0] += 1

        q_sb = qkvp.tile([P, N], FP, tag="q")
        k_sb = qkvp.tile([P, M], FP, tag="k")
        v_sb = qkvp.tile([P, M], FP, tag="v")
        exp_warm = consts.tile([P, 1], FP, name="exp_warm")
        HM = M // 2

        def proj(wname, bname, ins, out_sb):
            w0, w1 = w_tiles[wname]
            bt = b_tiles[bname]
            for hf in range(2):
                ps = sps.tile([P, HM], FP, tag="sc", name="pp")
                for nf in range(2):
                    sl = slice(nf * 512, (nf + 1) * 512)
                    osl = slice(hf * HM + nf * 512, hf * HM + (nf + 1) * 512)
                    nc.tensor.matmul(
                        ps[:, sl], lhsT=w0[:], rhs=ins[0][:, osl],
                        start=True, stop=False, skip_group_check=True,
                    )
                    nc.tensor.matmul(
                        ps[:, sl], lhsT=w1[:], rhs=ins[1][:, osl],
                        start=False, stop=True, skip_group_check=True,
                    )
                nc.scalar.activation(
                    out=out_sb[:, hf * HM : (hf + 1) * HM], in_=ps[:],
                    func=AF.Identity, bias=bt[:],
                )

        junk_absorb([identity])
        warm_ps = sps.tile([P, HM0], FP, tag="sc", name="warm")
        for _ in range(64):
            nc.tensor.matmul(
                warm_ps[:, 0:128], lhsT=identity[:], rhs=identity[:],
                start=True, stop=True, skip_group_check=True,
            )
        junk_absorb(stage_a)
        # warm the ACT exp/ln table while PE chews on the projections
        nc.scalar.activation(out=exp_warm[:], in_=b_tiles["bk"][:], func=AF.Exp)
        proj("wk", "bk", src_sb, k_sb)
        junk_absorb(stage_b)
        proj("wq", "bq", x_sb, q_sb)
        junk_absorb(stage_c)
        # v-projection in fp32r: 1 cycle/row on PE (vs 4 for fp32) and exempt
        # from the p-state ramp penalty; v is post-selection so the ~12-bit
        # mantissa cannot perturb the top-k set.
        wv_fr = []
        for i in range(2):
            tfr = wpool.tile([P, P], FR, tag=f"wvr{i}")
            nc.scalar.activation(out=tfr[:], in_=w_tiles["wv"][i][:], func=AF.Copy)
            wv_fr.append(tfr)
        src_fr = []
        for i in range(2):
            tfr = xpool.tile([P, M], FR, tag=f"sr{i}", name=f"sr{i}")
            nc.scalar.activation(out=tfr[:], in_=src_sb[i][:], func=AF.Copy)
            src_fr.append(tfr)
        w_tiles["wvr"] = (wv_fr[0], wv_fr[1])
        proj("wvr", "bv", src_fr, v_sb)
        # Read the junk tile on ACT so any later PE instruction recycling this
        # PSUM slot waits on the Activation sem (its one allowed wait).
        junk_sink = consts.tile([1, len(all_loaded)], FP, name="junk_sink")
        nc.scalar.activation(
            out=junk_sink[:], in_=junk[0:1, 0 : len(all_loaded)],
            func=AF.Copy,
        )
        xpool_cm.__exit__(None, None, None)

        # ---- vT per head (fp16): vT_sb[h] cols mt*64.. = v_h[:, mt*128..].T
        vT_sb = [
            vtp.tile([P, MT * DIM], HF, tag=f"vT{h}", name=f"vT{h}") for h in range(2)
        ]

        def emit_vt_build():
            for h in range(2):
                hs = slice(h * DIM, (h + 1) * DIM)
                for mt in range(MT):
                    tp = tps.tile([P, P], FP, tag="ptp", name="vtp")
                    nc.tensor.transpose(
                        tp[0:P, 0:DIM], v_sb[hs, mt * P : (mt + 1) * P],
                        identity[hs, hs],
                    )
                    nc.scalar.activation(
                        out=vT_sb[h][:, mt * DIM : (mt + 1) * DIM],
                        in_=tp[0:P, 0:DIM], func=AF.Copy,
                    )

        # ---- main loop: stage-skewed software pipeline over 32 tiles ----
        # slot t stages: scores(t) -> peel(t) -> den(t-1) -> rden/ln/e(t-2)
        #   -> select(t-3, GpSimd) -> transpose-DMA(t-4) -> AV chunks (PE)
        scp = ctx.enter_context(tc.tile_pool(name="ssb", bufs=5))
        srp = ctx.enter_context(tc.tile_pool(name="scr", bufs=5))
        epool = ctx.enter_context(tc.tile_pool(name="e", bufs=3))
        mkp = ctx.enter_context(tc.tile_pool(name="mk", bufs=3))
        pnp = ctx.enter_context(tc.tile_pool(name="pn", bufs=3))
        ptp = ctx.enter_context(tc.tile_pool(name="pt", bufs=2))
        m32p = ctx.enter_context(tc.tile_pool(name="m32", bufs=6))
        dpool = ctx.enter_context(tc.tile_pool(name="den", bufs=4))
        mgp = ctx.enter_context(tc.tile_pool(name="mg", bufs=2))
        NEG = -1.0e30
        ST = 4  # n-tiles per super-tile (phase)

        phases = [(st, h) for st in range(NT // ST) for h in range(2)]
        NTILES = len(phases) * ST
        tstate = {}  # per-tile tiles for the skewed stages
        pstate = {}  # per-phase: pTph tile, av psum tile
        mgs = {}  # st -> mg_sb tile
        av_fifo = []
        pend_phase_end = []

        def st_scores(t):
            p, ntl = t // ST, t % ST
            st, h = phases[p]
            hs = slice(h * DIM, (h + 1) * DIM)
            nn0 = (st * ST + ntl) * P
            s_sb = scp.tile([P, M], FP, tag="ssb", name="s_sb")
            for half in range(2):
                sp = sps.tile([P, M // 2], FP, tag="sc", name="sp")
                for mf in range(2):
                    sl = slice(mf * 512, (mf + 1) * 512)
                    msl = slice(half * 1024 + mf * 512, half * 1024 + (mf + 1) * 512)
                    nc.tensor.matmul(
                        sp[:, sl], lhsT=q_sb[hs, nn0 : nn0 + P], rhs=k_sb[hs, msl],
                        start=True, stop=True, skip_group_check=True,
                    )
                nc.scalar.activation(
                    out=s_sb[:, half * 1024 : (half + 1) * 1024], in_=sp[:],
                    func=AF.Copy,
                )
            tstate[t] = dict(s_sb=s_sb)

        def st_peel(t):
            # top-k peel on raw fp32 scores: 4x max8 + 3x match_replace
            s_sb = tstate[t]["s_sb"]
            scratch = srp.tile([P, M], FP, tag="scratch", name="scratch")
            m32 = m32p.tile([P, 8 * nrounds], FP, tag="m32", name="m32")
            src_t = s_sb
            for r in range(nrounds):
                m8 = m32[:, r * 8 : (r + 1) * 8]
                nc.vector.max(out=m8, in_=src_t[:])
                if r == nrounds - 1 and rem < 8:
                    nc.vector.memset(m8[:, rem:], NEG)
                nc.vector.match_replace(
                    out=scratch[:], in_to_replace=m8, in_values=src_t[:],
                    imm_value=NEG,
                )
                src_t = scratch
            tstate[t]["m32"] = m32
            tstate[t]["scratch"] = scratch

        def st_den(t):
            # ACT: den = sum(exp(scale * m32)) via accumulate output
            stt = tstate[t]
            e32 = dpool.tile([P, 8 * nrounds], FP, tag="e32", name="e32")
            den = dpool.tile([P, 1], FP, tag="den", name="den")
            nc.scalar.activation(
                out=e32[:], in_=stt["m32"][:], func=AF.Exp, scale=float(SCALE),
                accum_out=den[:],
            )
            stt["den"] = den

        def st_recip(t):
            stt = tstate[t]
            rden = dpool.tile([P, 1], FP, tag="rden", name="rden")
            nc.vector.reciprocal(rden[:], stt["den"][:])
            stt["rden"] = rden

        def st_exp(t):
            # ACT: nlden = ln(1/den); e = exp(scale*s + nlden) and
            # emk = exp(scale*scratch + nlden).  Same LUT args except at the
            # exactly-k top positions (scratch = NEG there), so e - emk is
            # exactly zero off-support and the normalized weight on it.
            stt = tstate[t]
            nlden = dpool.tile([P, 1], FP, tag="nlden", name="nlden")
            nc.scalar.activation(out=nlden[:], in_=stt["rden"][:], func=AF.Ln)
            # e/emk as separate half tiles so each half's subtract + transpose
            # DMA can launch as soon as its own exponentials land
            es, emks = [], []
            for hh in range(2):
                sl = slice(hh * (M // 2), (hh + 1) * (M // 2))
                e_sb = epool.tile([P, M // 2], HF, tag=f"e{hh}", name="e")
                nc.scalar.activation(
                    out=e_sb[:], in_=stt["s_sb"][:, sl], func=AF.Exp,
                    scale=float(SCALE), bias=nlden[:],
                )
                emk = mkp.tile([P, M // 2], HF, tag=f"emk{hh}", name="emk")
                nc.scalar.activation(
                    out=emk[:], in_=stt["scratch"][:, sl], func=AF.Exp,
                    scale=float(SCALE), bias=nlden[:],
                )
                es.append(e_sb)
                emks.append(emk)
            stt["e_sb"] = es
            stt["emk"] = emks

        def st_select(t):
            # GpSimd: pn = e - emk (normalized weights, exactly-k support),
            # in halves so each half transpose-DMA can launch ~2us earlier
            stt = tstate[t]
            pns = []
            for hh in range(2):
                pn = pnp.tile([P, M // 2], HF, tag=f"pn{hh}", name="pn")
                nc.gpsimd.tensor_sub(pn[:], stt["e_sb"][hh][:], stt["emk"][hh][:])
                pns.append(pn)
            stt["pn"] = pns

        def st_dmat(t):
            # One batched transpose-DMA of the whole [128, 2048] pn tile into
            # the per-phase pT layout [m_local, mt, n]; alternate the issue
            # queue between Sync and Activation to halve queue pressure.
            p, ntl = t // ST, t % ST
            if ntl == 0:
                pstate[p] = dict(
                    pT=ptp.tile([P, MT, ST * P], HF, tag="pT", name="pT")
                )
            pT = pstate[p]["pT"]
            pn = tstate[t]["pn"]
            for hh in range(2):
                nc.sync.dma_start_transpose(
                    out=pT[:, hh * (MT // 2) : (hh + 1) * (MT // 2),
                           ntl * P : (ntl + 1) * P],
                    in_=pn[hh][:],
                )
            if ntl == ST - 1:
                slot_now = t + 5  # st_dmat(t) runs at slot t + 5
                av_fifo.extend((p, c, slot_now + c) for c in range(ST))

        def st_av_chunk(slot):
            if not av_fifo or av_fifo[0][2] > slot:
                return
            p, c, _ = av_fifo.pop(0)
            st, h = phases[p]
            ps = pstate[p]
            if c == 0:
                ps["av"] = tps.tile([DIM, ST * P], FP, tag="ptp", name="av")
            for mt in range(c * 4, c * 4 + 4):
                nc.tensor.matmul(
                    ps["av"][:], lhsT=vT_sb[h][:, mt * DIM : (mt + 1) * DIM],
                    rhs=ps["pT"][:, mt, :],
                    start=(mt == 0), stop=(mt == MT - 1),
                    skip_group_check=True,
                )
            if c == ST - 1:
                pend_phase_end.append((slot + 2, p))

        def _phase_end(p):
            # av -> mg_sb; on odd phases also the merge matmul + output DMA
            st, h = phases[p]
            hs = slice(h * DIM, (h + 1) * DIM)
            if h == 0:
                mgs[st] = mgp.tile([P, ST * P], FP, tag="mg", name="mg")
            mg_sb = mgs[st]
            nc.scalar.activation(out=mg_sb[hs, :], in_=pstate[p]["av"][:], func=AF.Copy)
            if h == 1:
                n0 = st * ST * P
                for oh in range(2):
                    mm = mgps.tile([P, ST * P], FP, tag="mm", name="mm")
                    nc.tensor.matmul(
                        mm[:], lhsT=wm_sb[:, oh * P : (oh + 1) * P], rhs=mg_sb[:],
                        start=True, stop=True, skip_group_check=True,
                    )
                    mo = mgp.tile([P, ST * P], FP, tag="mo", name="mo")
                    nc.scalar.activation(out=mo[:], in_=mm[:], func=AF.Copy)
                    nc.sync.dma_start(
                        out=part[oh * P : (oh + 1) * P, n0 : n0 + ST * P], in_=mo[:]
                    )

        for t in range(NTILES + 14):
            if t < NTILES:
                st_scores(t)
            if t == 2:
                emit_vt_build()
            if 0 <= t - 2 < NTILES:
                st_recip(t - 2)
            if t < NTILES:
                st_peel(t)
            if 0 <= t - 3 < NTILES:
                st_exp(t - 3)
            if 0 <= t - 1 < NTILES:
                st_den(t - 1)
            if 0 <= t - 4 < NTILES:
                st_select(t - 4)
            if 0 <= t - 5 < NTILES:
                st_dmat(t - 5)
            st_av_chunk(t)
            while pend_phase_end and pend_phase_end[0][0] <= t:
                _phase_end(pend_phase_end.pop(0)[1])
        while pend_phase_end:
            _phase_end(pend_phase_end.pop(0)[1])

    import json as _json

    d = _json.loads(nc.to_json_bytes())
    _legalize_sync_waits(d)
    blob = _json.dumps(d).encode()
    nc.to_json_bytes = lambda: blob  # shadow the method; bass2jax serializes via this
    return nc


_PROGRAM_CACHE: dict[int, object] = {}
LAST_RESULTS = None


def _channel_order(hp: int) -> list[int]:
    # head-major, d-major within head: channels of head h are {4d + h}
    return [4 * d + 2 * hp + j for j in (0, 1) for d in range(DIM)]


def make_in_maps(x, source, Wq, bq, Wk, bk, Wv, bv, Wm):
    in_maps = []
    for c in range(N_CORES):
        b = c // 2
        hp = c % 2
        ch = _channel_order(hp)
        in_maps.append(
            {
                "xb": np.ascontiguousarray(x[b], dtype=np.float32),
                "src": np.ascontiguousarray(source[b], dtype=np.float32),
                "wqT": np.ascontiguousarray(Wq[ch, :].T, dtype=np.float32),
                "wkT": np.ascontiguousarray(Wk[ch, :].T, dtype=np.float32),
                "wvT": np.ascontiguousarray(Wv[ch, :].T, dtype=np.float32),
                "wmT": np.ascontiguousarray(Wm[:, ch].T, dtype=np.float32),
                "bq": np.ascontiguousarray(bq[ch].reshape(P, 1), dtype=np.float32),
                "bk": np.ascontiguousarray(bk[ch].reshape(P, 1), dtype=np.float32),
                "bv": np.ascontiguousarray(bv[ch].reshape(P, 1), dtype=np.float32),
            }
        )
    return in_maps


class _CompiledProgram:
    """Builds the Bass program once and caches the jitted shard_map callable
    (mirrors the multi-core branch of bass2jax.run_bass_via_pjrt)."""

    def __init__(self, k: int):
        import jax
        from jax.sharding import Mesh, PartitionSpec
        from jax.experimental.shard_map import shard_map
        from concourse import bass2jax

        bass2jax.install_neuronx_cc_hook()
        nc = build_program(k)
        self.nc = nc
        import concourse.mybir as _mybir

        in_names, out_names, out_avals, zero_outs = [], [], [], []
        for alloc in nc.m.functions[0].allocations:
            if not isinstance(alloc, _mybir.MemoryLocationSet):
                continue
            name = alloc.memorylocations[0].name
            partition_name = (
                nc.partition_id_tensor.name if nc.partition_id_tensor else None
            )
            if alloc.kind == "ExternalInput":
                if name != partition_name:
                    in_names.append(name)
            elif alloc.kind == "ExternalOutput":
                out_names.append(name)
                shape = tuple(alloc.tensor_shape)
                dtype = _mybir.dt.np(alloc.dtype)
                out_avals.append(jax.core.ShapedArray(shape, dtype))
                zero_outs.append(np.zeros(shape, dtype))
        self.in_names = list(in_names)
        self.out_names = out_names
        n_params = len(in_names)
        n_outs = len(out_avals)
        in_names = in_names + out_names
        self.in_names = self.in_names[:n_params]
        donate = tuple(range(n_params, n_params + n_outs))
        self.zero_outs = zero_outs
        self.out_avals = out_avals

        partition_name = (
            nc.partition_id_tensor.name if nc.partition_id_tensor else None
        )
        if partition_name is not None:
            in_names = in_names + [partition_name]

        def _body(*args):
            operands = list(args)
            if partition_name is not None:
                operands.append(bass2jax.partition_id_tensor())
            outs = bass2jax._bass_exec_p.bind(
                *operands,
                out_avals=tuple(out_avals),
                in_names=tuple(in_names),
                out_names=tuple(out_names),
                lowering_input_output_aliases=(),
                sim_require_finite=True,
                sim_require_nnan=True,
                nc=nc,
            )
            return tuple(outs)

        devices = jax.devices()[:N_CORES]
        mesh = Mesh(np.asarray(devices), ("core",))
        in_specs = (PartitionSpec("core"),) * (n_params + n_outs)
        out_specs = (PartitionSpec("core"),) * len(out_names)
        self.sharded = jax.jit(
            shard_map(
                _body, mesh=mesh, in_specs=in_specs, out_specs=out_specs,
                check_rep=False,
            ),
            donate_argnums=donate,
            keep_unused=True,
        )
        self.jax = jax

    def run(self, in_maps):
        np_in = [
            np.concatenate([np.asarray(m[name]) for m in in_maps], axis=0)
            for name in self.in_names
        ]
        zeros = [
            np.zeros((N_CORES * z.shape[0], *z.shape[1:]), z.dtype)
            for z in self.zero_outs
        ]
        out_arrs = self.jax.block_until_ready(self.sharded(*np_in, *zeros))
        return [
            {
                name: np.asarray(out_arrs[i]).reshape(
                    N_CORES, *self.out_avals[i].shape
                )[c]
                for i, name in enumerate(self.out_names)
            }
            for c in range(N_CORES)
        ]


def _get_program(k: int) -> _CompiledProgram:
    prog = _PROGRAM_CACHE.get(k)
    if prog is None:
        prog = _CompiledProgram(k)
        _PROGRAM_CACHE[k] = prog
    return prog


def kernel(x, source, Wq, bq, Wk, bk, Wv, bv, Wm, bm, k):
    global LAST_RESULTS
    k = int(k)
    x = np.asarray(x, dtype=np.float32)
    source = np.asarray(source, dtype=np.float32)
    prog = _get_program(k)
    in_maps = make_in_maps(x, source, Wq, bq, Wk, bk, Wv, bv, Wm)
    results = prog.run(in_maps)
    LAST_RESULTS = results
    out = np.zeros((B, D, N), dtype=np.float32)
    for c in range(N_CORES):
        out[c // 2] += results[c]["part"]
    out += np.asarray(bm, dtype=np.float32)[None, :, None]
    return out


# revision 39
# speedup vs baseline: 1.0023x; 1.0023x over previous
"""Trainium2 Bass kernel for sparse (top-k) multi-headed attention.

Problem shapes (hardcoded):
  x, source: [B=4, D=256, N=M=2048] f32
  Wq/Wk/Wv/Wm: [256, 256], bq/bk/bv/bm: [256], k=32 (top-k), H=4 heads, dim=64.

Sharding: 8 cores; core c handles batch b=c//2 and head pair hp=c%2
(heads 2hp, 2hp+1).  Channel c of D maps to (d, h) = (c//4, c%4) per the
reference reshape(B, dim, H, N).  The host reorders each core's 128
channels head-major/d-major so each head occupies 64 contiguous SBUF
partitions.  Each core returns its partial merge
  part = Wm[:, ch].T? -> out_part[o, n] = sum_{i in ch} Wm[o, i] * merged[i, n]
and the host sums the two partials per batch and adds bm.

Stage-skewed software pipeline, one slot per tile (tile = 128 query rows
x 2048 keys, 32 tiles/core); stage X(t-k) is emitted in slot t so every
engine stream stays dependency-clean (in-order engines, so a stage that
waits head-of-line-blocks everything behind it):
  slot t   PE : 4 fp32 score matmuls -> two [128,1024] PSUM tiles
           ACT: s_sb = copy(PSUM) per half (frees PSUM banks early)
           DVE: top-k peel = 4x (max8 + match_replace), exactly-k even
                with duplicate scores (match_replace replaces one
                instance per matched value)
  t-1      ACT: den = accum_out of exp(scale*m32) (one instruction)
  t-2      DVE: rden = 1/den
  t-3      ACT: nlden = Ln(rden); e = exp(scale*s + nlden) and
                emk = exp(scale*scratch + nlden), both fp16: identical
                LUT args except at the k top positions, so e - emk is
                exactly 0 off-support / the normalized weight on it
  t-4      GPS: pn = e - emk  (GpSimd, keeps it off the busy DVE/ACT)
  t-5      DMA: one batched XBAR transpose-DMA of pn into the per-phase
                pT layout [m_local, mt, n]  (replaces 512 PE transpose
                matmuls + 128 ACT PSUM->SBUF copies)
  fifo     PE : AV accumulation in fp16 (4 matmuls per slot), av psum
                double-buffered via the tps ring; merge matmul + output
                DMA per super-tile, phase-end copies deferred 2 slots so
                ACT never waits on PE's AV matmuls.
"""

import os
import sys

import ml_dtypes
import numpy as np

for _p in ("/opt/trn_rl_repo",):
    if _p not in sys.path and os.path.isdir(_p):
        sys.path.insert(0, _p)

import concourse.bass as bass
import concourse.mybir as mybir
import concourse.tile as tile
from concourse.bass_utils import run_bass_kernel_spmd
from concourse.masks import make_identity

B, D, N, M = 4, 256, 2048, 2048
H = 4
DIM = D // H  # 64
P = 128
NT = N // P  # n-tiles of 128 rows
MT = M // P  # m-tiles of 128 cols
SCALE = 1.0 / float(np.sqrt(DIM))  # 0.125
N_CORES = 8

FP = mybir.dt.float32
FR = mybir.dt.float32r
BF = mybir.dt.bfloat16
HF = mybir.dt.float16
A = mybir.AluOpType
AF = mybir.ActivationFunctionType


def _legalize_sync_waits(bir: dict) -> dict:
    """Split multi-wait instructions: walrus codegen allows only ONE sync wait
    per engine instruction (PE is HW-decoded; ACT/CTRL structs are just as
    limited).  Insert single-wait NoOps on the same engine immediately before
    any instruction carrying more than one wait; each NoOp takes one wait, the
    original keeps the last wait plus its updates."""
    nid = [0]
    for fn in bir["functions"]:
        for blk in fn["blocks"]:
            out = []
            for ins in blk["instructions"]:
                si = ins.get("sync_info")
                waits = (si or {}).get("on_wait") or []
                if len(waits) > 1:
                    for w in waits[:-1]:
                        nid[0] += 1
                        out.append(
                            {
                                "engine": ins["engine"],
                                "ins": [],
                                "name": f"{ins['name']}-sw{nid[0]}",
                                "opcode": "NoOp",
                                "outs": [],
                                "sync_info": {"on_update": [], "on_wait": [w]},
                            }
                        )
                    si["on_wait"] = [waits[-1]]
                out.append(ins)
            blk["instructions"] = out
    return bir


def build_program(k: int) -> bass.Bass:
    nrounds = (k + 7) // 8
    rem = k - (nrounds - 1) * 8  # valid slots in the last round (1..8)

    nc = bass.Bass(
        "TRN2",
        target_bir_lowering=False,
        debug=False,
        enable_asserts=True,
        num_devices=N_CORES,
    )

    # DRAM parameters (per-core shards, prepared by the host)
    xb = nc.dram_tensor("xb", [D, N], FP, kind="ExternalInput").ap()
    src = nc.dram_tensor("src", [D, M], FP, kind="ExternalInput").ap()
    wqT = nc.dram_tensor("wqT", [D, P], FP, kind="ExternalInput").ap()
    wkT = nc.dram_tensor("wkT", [D, P], FP, kind="ExternalInput").ap()
    wvT = nc.dram_tensor("wvT", [D, P], FP, kind="ExternalInput").ap()
    wmT = nc.dram_tensor("wmT", [P, D], FP, kind="ExternalInput").ap()
    bqv = nc.dram_tensor("bq", [P, 1], FP, kind="ExternalInput").ap()
    bkv = nc.dram_tensor("bk", [P, 1], FP, kind="ExternalInput").ap()
    bvv = nc.dram_tensor("bv", [P, 1], FP, kind="ExternalInput").ap()
    part = nc.dram_tensor("part", [D, N], FP, kind="ExternalOutput").ap()

    from contextlib import ExitStack

    with tile.TileContext(nc) as tc, ExitStack() as ctx:
        consts = ctx.enter_context(tc.tile_pool(name="consts", bufs=1))
        wpool = ctx.enter_context(tc.tile_pool(name="w", bufs=1))
        qkvp = ctx.enter_context(tc.tile_pool(name="qkv", bufs=1))
        vtp = ctx.enter_context(tc.tile_pool(name="vt", bufs=1))
        xpool_cm = tc.tile_pool(name="x", bufs=1)
        xpool = xpool_cm.__enter__()

        identity = consts.tile([P, P], FP)
        make_identity(nc, identity)

        # ---- load weights / biases / activations ----
        # k-path tensors on the Sync DGE queue, q-path on the Activation DGE
        # queue so the two transfer streams overlap; wv/wm later on Sync.
        w_tiles = {}
        for name, ap, eng in (
            ("wk", wkT, nc.sync), ("wq", wqT, nc.scalar), ("wv", wvT, nc.sync)
        ):
            t0 = wpool.tile([P, P], FP, tag=name + "0")
            t1 = wpool.tile([P, P], FP, tag=name + "1")
            eng.dma_start(out=t0[:], in_=ap[0:P, :])
            eng.dma_start(out=t1[:], in_=ap[P : 2 * P, :])
            w_tiles[name] = (t0, t1)
        wm_sb = wpool.tile([P, D], FP, tag="wm")
        nc.sync.dma_start(out=wm_sb[:], in_=wmT[:, :])
        b_tiles = {}
        for name, ap, eng in (
            ("bq", bqv, nc.scalar), ("bk", bkv, nc.sync), ("bv", bvv, nc.sync)
        ):
            t = wpool.tile([P, 1], FP, tag=name)
            eng.dma_start(out=t[:], in_=ap[:, :])
            b_tiles[name] = t

        x_sb = [xpool.tile([P, N], FP, tag=f"x{i}", name=f"x{i}") for i in range(2)]
        src_sb = [xpool.tile([P, M], FP, tag=f"s{i}", name=f"s{i}") for i in range(2)]
        for i in range(2):
            nc.scalar.dma_start(out=x_sb[i][:], in_=xb[i * P : (i + 1) * P, :])
            nc.sync.dma_start(out=src_sb[i][:], in_=src[i * P : (i + 1) * P, :])

        # Persistent PSUM pools (8 banks total, never recycled across phases):
        #   sps: one [P, 2048] fp32 score tile  = 4 banks
        #   tps: junk / vT transposes / p-transposes (bufs=2)
        #   avps: AV accumulator [64, 512]      = 1 bank
        #   mgps: merge matmul [128, 512]       = 1 bank
        sps = ctx.enter_context(tc.tile_pool(name="sps", bufs=2, space="PSUM"))
        tps = ctx.enter_context(tc.tile_pool(name="tps", bufs=2, space="PSUM"))
        mgps = ctx.enter_context(tc.tile_pool(name="mgps", bufs=2, space="PSUM"))

        # Absorb DMA-completion semaphores into PE's observed clock in three
        # stages so the k-projection starts as soon as its own inputs landed:
        # one tiny single-wait matmul per loaded tile, so no later PE
        # instruction needs a second fresh wait.
        stage_a = [w_tiles["wk"][0], w_tiles["wk"][1], b_tiles["bk"]] + src_sb
        stage_b = [w_tiles["wq"][0], w_tiles["wq"][1], b_tiles["bq"]] + x_sb
        stage_c = [
            w_tiles["wv"][0], w_tiles["wv"][1], b_tiles["bv"], wm_sb
        ]
        all_loaded = [identity] + stage_a + stage_b + stage_c
        HM0 = M // 2
        junk = tps.tile([P, P], FP, tag="ptp", name="junk")
        ji = [0]

        def junk_absorb(tiles):
            for t in tiles:
                nc.tensor.matmul(
                    junk[0:1, ji[0] : ji[0] + 1], lhsT=t[:, 0:1], rhs=t[:, 0:1],
                    start=True, stop=True, skip_group_check=True,
                )
                ji[0] += 1

        q_sb = qkvp.tile([P, N], FP, tag="q")
        k_sb = qkvp.tile([P, M], FP, tag="k")
        v_sb = qkvp.tile([P, M], FP, tag="v")
        exp_warm = consts.tile([P, 1], FP, name="exp_warm")
        HM = M // 2

        def proj(wname, bname, ins, out_sb):
            w0, w1 = w_tiles[wname]
            bt = b_tiles[bname]
            for hf in range(2):
                ps = sps.tile([P, HM], FP, tag="sc", name="pp")
                for nf in range(2):
                    sl = slice(nf * 512, (nf + 1) * 512)
                    osl = slice(hf * HM + nf * 512, hf * HM + (nf + 1) * 512)
                    nc.tensor.matmul(
                        ps[:, sl], lhsT=w0[:], rhs=ins[0][:, osl],
                        start=True, stop=False, skip_group_check=True,
                    )
                    nc.tensor.matmul(
                        ps[:, sl], lhsT=w1[:], rhs=ins[1][:, osl],
                        start=False, stop=True, skip_group_check=True,
                    )
                nc.scalar.activation(
                    out=out_sb[:, hf * HM : (hf + 1) * HM], in_=ps[:],
                    func=AF.Identity, bias=bt[:],
                )

        junk_absorb([identity])
        warm_ps = sps.tile([P, HM0], FP, tag="sc", name="warm")
        for _ in range(64):
            nc.tensor.matmul(
                warm_ps[:, 0:128], lhsT=identity[:], rhs=identity[:],
                start=True, stop=True, skip_group_check=True,
            )
        junk_absorb(stage_a)
        # warm the ACT exp/ln table while PE chews on the projections
        nc.scalar.activation(out=exp_warm[:], in_=b_tiles["bk"][:], func=AF.Exp)
        proj("wk", "bk", src_sb, k_sb)
        junk_absorb(stage_b)
        proj("wq", "bq", x_sb, q_sb)
        junk_absorb(stage_c)
        # v-projection in fp32r: 1 cycle/row on PE (vs 4 for fp32) and exempt
        # from the p-state ramp penalty; v is post-selection so the ~12-bit
        # mantissa cannot perturb the top-k set.
        wv_fr = []
        for i in range(2):
            tfr = wpool.tile([P, P], FR, tag=f"wvr{i}")
            nc.scalar.activation(out=tfr[:], in_=w_tiles["wv"][i][:], func=AF.Copy)
            wv_fr.append(tfr)
        src_fr = []
        for i in range(2):
            tfr = xpool.tile([P, M], FR, tag=f"sr{i}", name=f"sr{i}")
            nc.scalar.activation(out=tfr[:], in_=src_sb[i][:], func=AF.Copy)
            src_fr.append(tfr)
        w_tiles["wvr"] = (wv_fr[0], wv_fr[1])
        proj("wvr", "bv", src_fr, v_sb)
        # Read the junk tile on ACT so any later PE instruction recycling this
        # PSUM slot waits on the Activation sem (its one allowed wait).
        junk_sink = consts.tile([1, len(all_loaded)], FP, name="junk_sink")
        nc.scalar.activation(
            out=junk_sink[:], in_=junk[0:1, 0 : len(all_loaded)],
            func=AF.Copy,
        )
        xpool_cm.__exit__(None, None, None)

        # ---- vT per head (fp16): vT_sb[h] cols mt*64.. = v_h[:, mt*128..].T
        vT_sb = [
            vtp.tile([P, MT * DIM], HF, tag=f"vT{h}", name=f"vT{h}") for h in range(2)
        ]

        def emit_vt_build():
            for h in range(2):
                hs = slice(h * DIM, (h + 1) * DIM)
                for mt in range(MT):
                    tp = tps.tile([P, P], FP, tag="ptp", name="vtp")
                    nc.tensor.transpose(
                        tp[0:P, 0:DIM], v_sb[hs, mt * P : (mt + 1) * P],
                        identity[hs, hs],
                    )
                    nc.scalar.activation(
                        out=vT_sb[h][:, mt * DIM : (mt + 1) * DIM],
                        in_=tp[0:P, 0:DIM], func=AF.Copy,
                    )

        # ---- main loop: stage-skewed software pipeline over 32 tiles ----
        # slot t stages: scores(t) -> peel(t) -> den(t-1) -> rden/ln/e(t-2)
        #   -> select(t-3, GpSimd) -> transpose-DMA(t-4) -> AV chunks (PE)
        scp = ctx.enter_context(tc.tile_pool(name="ssb", bufs=5))
        srp = ctx.enter_context(tc.tile_pool(name="scr", bufs=5))
        epool = ctx.enter_context(tc.tile_pool(name="e", bufs=3))
        mkp = ctx.enter_context(tc.tile_pool(name="mk", bufs=3))
        pnp = ctx.enter_context(tc.tile_pool(name="pn", bufs=3))
        ptp = ctx.enter_context(tc.tile_pool(name="pt", bufs=2))
        m32p = ctx.enter_context(tc.tile_pool(name="m32", bufs=6))
        dpool = ctx.enter_context(tc.tile_pool(name="den", bufs=4))
        mgp = ctx.enter_context(tc.tile_pool(name="mg", bufs=2))
        NEG = -1.0e30
        ST = 4  # n-tiles per super-tile (phase)

        phases = [(st, h) for st in range(NT // ST) for h in range(2)]
        NTILES = len(phases) * ST
        tstate = {}  # per-tile tiles for the skewed stages
        pstate = {}  # per-phase: pTph tile, av psum tile
        mgs = {}  # st -> mg_sb tile
        av_fifo = []
        pend_phase_end = []

        def st_scores(t):
            p, ntl = t // ST, t % ST
            st, h = phases[p]
            hs = slice(h * DIM, (h + 1) * DIM)
            nn0 = (st * ST + ntl) * P
            s_sb = scp.tile([P, M], FP, tag="ssb", name="s_sb")
            for half in range(2):
                sp = sps.tile([P, M // 2], FP, tag="sc", name="sp")
                for mf in range(2):
                    sl = slice(mf * 512, (mf + 1) * 512)
                    msl = slice(half * 1024 + mf * 512, half * 1024 + (mf + 1) * 512)
                    nc.tensor.matmul(
                        sp[:, sl], lhsT=q_sb[hs, nn0 : nn0 + P], rhs=k_sb[hs, msl],
                        start=True, stop=True, skip_group_check=True,
                    )
                nc.scalar.activation(
                    out=s_sb[:, half * 1024 : (half + 1) * 1024], in_=sp[:],
                    func=AF.Copy,
                )
            tstate[t] = dict(s_sb=s_sb)

        def st_peel(t):
            # top-k peel on raw fp32 scores: 4x max8 + 3x match_replace
            s_sb = tstate[t]["s_sb"]
            scratch = srp.tile([P, M], FP, tag="scratch", name="scratch")
            m32 = m32p.tile([P, 8 * nrounds], FP, tag="m32", name="m32")
            src_t = s_sb
            for r in range(nrounds):
                m8 = m32[:, r * 8 : (r + 1) * 8]
                nc.vector.max(out=m8, in_=src_t[:])
                if r == nrounds - 1 and rem < 8:
                    nc.vector.memset(m8[:, rem:], NEG)
                nc.vector.match_replace(
                    out=scratch[:], in_to_replace=m8, in_values=src_t[:],
                    imm_value=NEG,
                )
                src_t = scratch
            tstate[t]["m32"] = m32
            tstate[t]["scratch"] = scratch

        def st_den(t):
            # ACT: den = sum(exp(scale * m32)) via accumulate output
            stt = tstate[t]
            e32 = dpool.tile([P, 8 * nrounds], FP, tag="e32", name="e32")
            den = dpool.tile([P, 1], FP, tag="den", name="den")
            nc.scalar.activation(
                out=e32[:], in_=stt["m32"][:], func=AF.Exp, scale=float(SCALE),
                accum_out=den[:],
            )
            stt["den"] = den

        def st_recip(t):
            stt = tstate[t]
            rden = dpool.tile([P, 1], FP, tag="rden", name="rden")
            nc.vector.reciprocal(rden[:], stt["den"][:])
            stt["rden"] = rden

        def st_exp(t):
            # ACT: nlden = ln(1/den); e = exp(scale*s + nlden) and
            # emk = exp(scale*scratch + nlden).  Same LUT args except at the
            # exactly-k top positions (scratch = NEG there), so e - emk is
            # exactly zero off-support and the normalized weight on it.
            stt = tstate[t]
            nlden = dpool.tile([P, 1], FP, tag="nlden", name="nlden")
            nc.scalar.activation(out=nlden[:], in_=stt["rden"][:], func=AF.Ln)
            # e/emk as separate half tiles so each half's subtract + transpose
            # DMA can launch as soon as its own exponentials land
            es, emks = [], []
            for hh in range(2):
                sl = slice(hh * (M // 2), (hh + 1) * (M // 2))
                e_sb = epool.tile([P, M // 2], HF, tag=f"e{hh}", name="e")
                nc.scalar.activation(
                    out=e_sb[:], in_=stt["s_sb"][:, sl], func=AF.Exp,
                    scale=float(SCALE), bias=nlden[:],
                )
                emk = mkp.tile([P, M // 2], HF, tag=f"emk{hh}", name="emk")
                nc.scalar.activation(
                    out=emk[:], in_=stt["scratch"][:, sl], func=AF.Exp,
                    scale=float(SCALE), bias=nlden[:],
                )
                es.append(e_sb)
                emks.append(emk)
            stt["e_sb"] = es
            stt["emk"] = emks

        def st_select(t):
            # GpSimd: pn = e - emk (normalized weights, exactly-k support),
            # in halves so each half transpose-DMA can launch ~2us earlier
            stt = tstate[t]
            pns = []
            for hh in range(2):
                pn = pnp.tile([P, M // 2], HF, tag=f"pn{hh}", name="pn")
                nc.gpsimd.tensor_sub(pn[:], stt["e_sb"][hh][:], stt["emk"][hh][:])
                pns.append(pn)
            stt["pn"] = pns

        def st_dmat(t):
            # One batched transpose-DMA of the whole [128, 2048] pn tile into
            # the per-phase pT layout [m_local, mt, n]; alternate the issue
            # queue between Sync and Activation to halve queue pressure.
            p, ntl = t // ST, t % ST
            if ntl == 0:
                pstate[p] = dict(
                    pT=ptp.tile([P, MT, ST * P], HF, tag="pT", name="pT")
                )
            pT = pstate[p]["pT"]
            pn = tstate[t]["pn"]
            for hh in range(2):
                nc.sync.dma_start_transpose(
                    out=pT[:, hh * (MT // 2) : (hh + 1) * (MT // 2),
                           ntl * P : (ntl + 1) * P],
                    in_=pn[hh][:],
                )
            if ntl == ST - 1:
                slot_now = t + 5  # st_dmat(t) runs at slot t + 5
                av_fifo.extend((p, c, slot_now + c) for c in range(ST))

        def st_av_chunk(slot):
            if not av_fifo or av_fifo[0][2] > slot:
                return
            p, c, _ = av_fifo.pop(0)
            st, h = phases[p]
            ps = pstate[p]
            if c == 0:
                ps["av"] = tps.tile([DIM, ST * P], FP, tag="ptp", name="av")
            for mt in range(c * 4, c * 4 + 4):
                nc.tensor.matmul(
                    ps["av"][:], lhsT=vT_sb[h][:, mt * DIM : (mt + 1) * DIM],
                    rhs=ps["pT"][:, mt, :],
                    start=(mt == 0), stop=(mt == MT - 1),
                    skip_group_check=True,
                )
            if c == ST - 1:
                pend_phase_end.append((slot + 1, p))

        def _phase_end(p):
            # av -> mg_sb; on odd phases also the merge matmul + output DMA
            st, h = phases[p]
            hs = slice(h * DIM, (h + 1) * DIM)
            if h == 0:
                mgs[st] = mgp.tile([P, ST * P], FP, tag="mg", name="mg")
            mg_sb = mgs[st]
            nc.scalar.activation(out=mg_sb[hs, :], in_=pstate[p]["av"][:], func=AF.Copy)
            if h == 1:
                n0 = st * ST * P
                for oh in range(2):
                    mm = mgps.tile([P, ST * P], FP, tag="mm", name="mm")
                    nc.tensor.matmul(
                        mm[:], lhsT=wm_sb[:, oh * P : (oh + 1) * P], rhs=mg_sb[:],
                        start=True, stop=True, skip_group_check=True,
                    )
                    mo = mgp.tile([P, ST * P], FP, tag="mo", name="mo")
                    nc.scalar.activation(out=mo[:], in_=mm[:], func=AF.Copy)
                    nc.sync.dma_start(
                        out=part[oh * P : (oh + 1) * P, n0 : n0 + ST * P], in_=mo[:]
                    )

        for t in range(NTILES + 14):
            if t < NTILES:
                st_scores(t)
            if t == 2:
                emit_vt_build()
            if 0 <= t - 2 < NTILES:
                st_recip(t - 2)
            if t < NTILES:
                st_peel(t)
            if 0 <= t - 3 < NTILES:
                st_exp(t - 3)
            if 0 <= t - 1 < NTILES:
                st_den(t - 1)
            if 0 <= t - 4 < NTILES:
                st_select(t - 4)
            if 0 <= t - 5 < NTILES:
                st_dmat(t - 5)
            st_av_chunk(t)
            while pend_phase_end and pend_phase_end[0][0] <= t:
                _phase_end(pend_phase_end.pop(0)[1])
        while pend_phase_end:
            _phase_end(pend_phase_end.pop(0)[1])

    import json as _json

    d = _json.loads(nc.to_json_bytes())
    _legalize_sync_waits(d)
    blob = _json.dumps(d).encode()
    nc.to_json_bytes = lambda: blob  # shadow the method; bass2jax serializes via this
    return nc


_PROGRAM_CACHE: dict[int, object] = {}
LAST_RESULTS = None


def _channel_order(hp: int) -> list[int]:
    # head-major, d-major within head: channels of head h are {4d + h}
    return [4 * d + 2 * hp + j for j in (0, 1) for d in range(DIM)]


def make_in_maps(x, source, Wq, bq, Wk, bk, Wv, bv, Wm):
    in_maps = []
    for c in range(N_CORES):
        b = c // 2
        hp = c % 2
        ch = _channel_order(hp)
        in_maps.append(
            {
                "xb": np.ascontiguousarray(x[b], dtype=np.float32),
                "src": np.ascontiguousarray(source[b], dtype=np.float32),
                "wqT": np.ascontiguousarray(Wq[ch, :].T, dtype=np.float32),
                "wkT": np.ascontiguousarray(Wk[ch, :].T, dtype=np.float32),
                "wvT": np.ascontiguousarray(Wv[ch, :].T, dtype=np.float32),
                "wmT": np.ascontiguousarray(Wm[:, ch].T, dtype=np.float32),
                "bq": np.ascontiguousarray(bq[ch].reshape(P, 1), dtype=np.float32),
                "bk": np.ascontiguousarray(bk[ch].reshape(P, 1), dtype=np.float32),
                "bv": np.ascontiguousarray(bv[ch].reshape(P, 1), dtype=np.float32),
            }
        )
    return in_maps


class _CompiledProgram:
    """Builds the Bass program once and caches the jitted shard_map callable
    (mirrors the multi-core branch of bass2jax.run_bass_via_pjrt)."""

    def __init__(self, k: int):
        import jax
        from jax.sharding import Mesh, PartitionSpec
        from jax.experimental.shard_map import shard_map
        from concourse import bass2jax

        bass2jax.install_neuronx_cc_hook()
        nc = build_program(k)
        self.nc = nc
        import concourse.mybir as _mybir

        in_names, out_names, out_avals, zero_outs = [], [], [], []
        for alloc in nc.m.functions[0].allocations:
            if not isinstance(alloc, _mybir.MemoryLocationSet):
                continue
            name = alloc.memorylocations[0].name
            partition_name = (
                nc.partition_id_tensor.name if nc.partition_id_tensor else None
            )
            if alloc.kind == "ExternalInput":
                if name != partition_name:
                    in_names.append(name)
            elif alloc.kind == "ExternalOutput":
                out_names.append(name)
                shape = tuple(alloc.tensor_shape)
                dtype = _mybir.dt.np(alloc.dtype)
                out_avals.append(jax.core.ShapedArray(shape, dtype))
                zero_outs.append(np.zeros(shape, dtype))
        self.in_names = list(in_names)
        self.out_names = out_names
        n_params = len(in_names)
        n_outs = len(out_avals)
        in_names = in_names + out_names
        self.in_names = self.in_names[:n_params]
        donate = tuple(range(n_params, n_params + n_outs))
        self.zero_outs = zero_outs
        self.out_avals = out_avals

        partition_name = (
            nc.partition_id_tensor.name if nc.partition_id_tensor else None
        )
        if partition_name is not None:
            in_names = in_names + [partition_name]

        def _body(*args):
            operands = list(args)
            if partition_name is not None:
                operands.append(bass2jax.partition_id_tensor())
            outs = bass2jax._bass_exec_p.bind(
                *operands,
                out_avals=tuple(out_avals),
                in_names=tuple(in_names),
                out_names=tuple(out_names),
                lowering_input_output_aliases=(),
                sim_require_finite=True,
                sim_require_nnan=True,
                nc=nc,
            )
            return tuple(outs)

        devices = jax.devices()[:N_CORES]
        mesh = Mesh(np.asarray(devices), ("core",))
        in_specs = (PartitionSpec("core"),) * (n_params + n_outs)
        out_specs = (PartitionSpec("core"),) * len(out_names)
        self.sharded = jax.jit(
            shard_map(
                _body, mesh=mesh, in_specs=in_specs, out_specs=out_specs,
                check_rep=False,
            ),
            donate_argnums=donate,
            keep_unused=True,
        )
        self.jax = jax

    def run(self, in_maps):
        np_in = [
            np.concatenate([np.asarray(m[name]) for m in in_maps], axis=0)
            for name in self.in_names
        ]
        zeros = [
            np.zeros((N_CORES * z.shape[0], *z.shape[1:]), z.dtype)
            for z in self.zero_outs
        ]
        out_arrs = self.jax.block_until_ready(self.sharded(*np_in, *zeros))
        return [
            {
                name: np.asarray(out_arrs[i]).reshape(
                    N_CORES, *self.out_avals[i].shape
                )[c]
                for i, name in enumerate(self.out_names)
            }
            for c in range(N_CORES)
        ]


def _get_program(k: int) -> _CompiledProgram:
    prog = _PROGRAM_CACHE.get(k)
    if prog is None:
        prog = _CompiledProgram(k)
        _PROGRAM_CACHE[k] = prog
    return prog


def kernel(x, source, Wq, bq, Wk, bk, Wv, bv, Wm, bm, k):
    global LAST_RESULTS
    k = int(k)
    x = np.asarray(x, dtype=np.float32)
    source = np.asarray(source, dtype=np.float32)
    prog = _get_program(k)
    in_maps = make_in_maps(x, source, Wq, bq, Wk, bk, Wv, bv, Wm)
    results = prog.run(in_maps)
    LAST_RESULTS = results
    out = np.zeros((B, D, N), dtype=np.float32)
    for c in range(N_CORES):
        out[c // 2] += results[c]["part"]
    out += np.asarray(bm, dtype=np.float32)[None, :, None]
    return out
